# revision 1
# baseline (speedup 1.0000x reference)
"""Trainium2 Bass kernel for a 2-layer GRU (B=256, S=1024, IN=4+META=4, H=256) + FC head.

Strategy (data-parallel over batch, 8 cores, 32 batch rows each):
  - All tensors on-chip live in a "transposed" layout: partition dim = 128
    hidden/gate units (chunked), free dim = batch (32), so DVE/ACT use all
    128 lanes.
  - Per GRU step: hg^T = W_hh @ h^T via 12 weight-stationary matmuls
    (6 gate-chunks x 2 K-chunks, N=32 moving cols of h^T), accumulating
    into one PSUM tile [128, 6, 32] (fp32).  Weights stream through
    LDWEIGHTS in bf16 (FWL).
  - Input projections xg = W_ih @ x (+ both biases) are computed per
    window of T steps as efficient GEMMs, evacuated PSUM->SBUF (bf16)
    on ScalarE (ACTIVATE Identity) with the bias folded per partition.
  - The two layers' scans run software-pipelined one window apart so their
    serial gate chains interleave on the engines.
  - Everything except PSUM accumulation and the gate adds is bf16.

kernel(**inputs) takes the FULL fp32 inputs, does numpy layout prep, runs
the SPMD program on 8 cores, and concatenates the per-core [32, 1] outputs.
"""

import numpy as np
import ml_dtypes
from contextlib import ExitStack

import concourse.bass as bass
import concourse.bacc as bacc
import concourse.tile as tile
import concourse.mybir as mybir
from concourse.bass_utils import run_bass_kernel_spmd

AF = mybir.ActivationFunctionType
BF16 = mybir.dt.bfloat16
F32 = mybir.dt.float32

B = 256
NCORES = 8
BL = B // NCORES  # 32 batch rows per core
S_FULL = 1024
H = 256
G = 3 * H  # 768
KIN = 8  # IN + META
NMCH = G // 128  # 6 gate chunks
NKCH = H // 128  # 2 hidden chunks


def build_program(S=S_FULL, T=64, evac_act_ratio=2):
    """Build the single-core SPMD Bass program.

    S: sequence length; T: window (steps per xg GEMM); both scans are
    emitted interleaved with scan1 lagging scan0 by one window.
    """
    assert S % T == 0 and (T * BL) % 512 == 0
    NW = S // T
    NCH = (T * BL) // 512  # 512-wide N-chunks per window GEMM
    SPC = 512 // BL  # steps per N-chunk (16)

    nc = bacc.Bacc()

    xinT_d = nc.declare_dram_parameter("xinT", [KIN, S * BL], BF16, False)
    wih0T_d = nc.declare_dram_parameter("wih0T", [KIN, G], BF16, False)
    whh0T_d = nc.declare_dram_parameter("whh0T", [128, NKCH, G], BF16, False)
    wih1T_d = nc.declare_dram_parameter("wih1T", [128, NKCH, G], BF16, False)
    whh1T_d = nc.declare_dram_parameter("whh1T", [128, NKCH, G], BF16, False)
    b0T_d = nc.declare_dram_parameter("b0T", [128, NMCH], F32, False)
    b1T_d = nc.declare_dram_parameter("b1T", [128, NMCH], F32, False)
    b0hn_d = nc.declare_dram_parameter("b0hn", [128, SPC * NKCH * BL], BF16, False)
    b1hn_d = nc.declare_dram_parameter("b1hn", [128, SPC * NKCH * BL], BF16, False)
    b0f_d = nc.declare_dram_parameter("b0f", [128, NMCH, SPC * BL], BF16, False)
    b1f_d = nc.declare_dram_parameter("b1f", [128, NMCH, SPC * BL], BF16, False)
    fcWT_d = nc.declare_dram_parameter("fcWT", [128, NKCH], BF16, False)
    fcb_d = nc.declare_dram_parameter("fcb", [BL, 1], F32, False)
    y_d = nc.declare_dram_parameter("y", [BL, 1], F32, True)

    evac_ctr = [0]

    with ExitStack() as ctx:
        tc = ctx.enter_context(tile.TileContext(nc))
        consts = ctx.enter_context(tc.tile_pool(name="consts", bufs=1))
        xinp = ctx.enter_context(tc.tile_pool(name="xinp", bufs=2))
        xgp = ctx.enter_context(tc.tile_pool(name="xgp", bufs=2 * NCH))
        h1p = ctx.enter_context(tc.tile_pool(name="h1p", bufs=2))
        gp = ctx.enter_context(tc.tile_pool(name="gp", bufs=4))
        h2p = ctx.enter_context(tc.tile_pool(name="h2p", bufs=4))
        psc = ctx.enter_context(tc.tile_pool(name="psc", bufs=2, space="PSUM"))
        psg = ctx.enter_context(tc.tile_pool(name="psg", bufs=3, space="PSUM"))

        # ---- constants ----
        whh0_sb = consts.tile([128, NKCH, G], BF16)
        nc.sync.dma_start(whh0_sb, whh0T_d[:, :, :])
        whh1_sb = consts.tile([128, NKCH, G], BF16)
        nc.sync.dma_start(whh1_sb, whh1T_d[:, :, :])
        wih1_sb = consts.tile([128, NKCH, G], BF16)
        nc.sync.dma_start(wih1_sb, wih1T_d[:, :, :])
        wih0_sb = consts.tile([KIN, G], BF16)
        nc.sync.dma_start(wih0_sb, wih0T_d[:, :])
        b0_sb = consts.tile([128, NMCH], F32)
        nc.sync.dma_start(b0_sb, b0T_d[:, :])
        b1_sb = consts.tile([128, NMCH], F32)
        nc.sync.dma_start(b1_sb, b1T_d[:, :])
        b0hn_sb = consts.tile([128, SPC, NKCH, BL], BF16)
        nc.sync.dma_start(b0hn_sb, b0hn_d[:, :].rearrange("p (s c b) -> p s c b", s=SPC, c=NKCH))
        b1hn_sb = consts.tile([128, SPC, NKCH, BL], BF16)
        nc.sync.dma_start(b1hn_sb, b1hn_d[:, :].rearrange("p (s c b) -> p s c b", s=SPC, c=NKCH))
        b0f_sb = consts.tile([128, NMCH, SPC, BL], BF16)
        nc.sync.dma_start(b0f_sb, b0f_d[:, :, :].rearrange("p m (s b) -> p m s b", s=SPC))
        b1f_sb = consts.tile([128, NMCH, SPC, BL], BF16)
        nc.sync.dma_start(b1f_sb, b1f_d[:, :, :].rearrange("p m (s b) -> p m s b", s=SPC))
        fcW_sb = consts.tile([128, NKCH], BF16)
        nc.sync.dma_start(fcW_sb, fcWT_d[:, :])
        fcb_sb = consts.tile([BL, 1], F32)
        nc.sync.dma_start(fcb_sb, fcb_d[:, :])
        zeros2 = consts.tile([128, NKCH, BL], BF16)
        nc.vector.memset(zeros2, 0.0)

        def evac(out_ap, psum_ap, bias_ap, bias_bcast_ap):
            """PSUM->SBUF move with bias add, alternating ScalarE/VectorE.
            DVE uses tensor_add with a broadcast-bias constant (TensorScalarPtr
            is rejected by walrus when Tile attaches >1 sync wait)."""
            evac_ctr[0] += 1
            if evac_ctr[0] % 2 == 0:
                nc.scalar.activation(out_ap, psum_ap, AF.Identity, bias=bias_ap)
            else:
                nc.vector.tensor_add(out_ap, psum_ap, bias_bcast_ap)

        def emit_xg_gemm0(xin_w):
            subs = []
            for nch in range(NCH):
                xg_sub = xgp.tile([128, SPC, 8, BL], BF16, tag="xg0")
                nc.vector.tensor_copy(xg_sub[:, :, 4:6, :], b0hn_sb)
                for m in range(NMCH):
                    P = psg.tile([128, SPC, BL], F32, tag="psg")
                    nc.tensor.matmul(
                        P,
                        wih0_sb[:, bass.ts(m, 128)],
                        xin_w[:, bass.ts(nch, 512)],
                        start=True,
                        stop=True,
                    )
                    evac(xg_sub[:, :, m if m < 4 else m + 2, :], P,
                         b0_sb[:, m : m + 1], b0f_sb[:, m, :, :])
                subs.append(xg_sub)
            return subs

        def emit_xg_gemm1(h1win):
            subs = []
            for nch in range(NCH):
                xg_sub = xgp.tile([128, SPC, 8, BL], BF16, tag="xg1")
                nc.vector.tensor_copy(xg_sub[:, :, 4:6, :], b1hn_sb)
                for m in range(NMCH):
                    P = psg.tile([128, SPC, BL], F32, tag="psg")
                    for kc in range(NKCH):
                        nc.tensor.matmul(
                            P,
                            wih1_sb[:, kc, bass.ts(m, 128)],
                            h1win[:, kc, bass.ts(nch, SPC), :],
                            start=(kc == 0),
                            stop=(kc == NKCH - 1),
                        )
                    evac(xg_sub[:, :, m if m < 4 else m + 2, :], P,
                         b1_sb[:, m : m + 1], b1f_sb[:, m, :, :])
                subs.append(xg_sub)
            return subs

        def emit_gru_step(tag, whh_sb, xg_sub, tl, hprev, hout):
            P = psc.tile([128, NMCH, BL], F32, tag="ps" + tag)
            for m in range(NMCH):
                for kc in range(NKCH):
                    nc.tensor.matmul(
                        P[:, m, :],
                        whh_sb[:, kc, bass.ts(m, 128)],
                        hprev[:, kc, :],
                        start=(kc == 0),
                        stop=(kc == NKCH - 1),
                    )
            xg_t = xg_sub[:, tl, :, :]
            # one add covers r/z gate pre-activations AND (hg_n + b_hn)
            a_all = gp.tile([128, 6, BL], BF16, tag=tag + "a_all")
            nc.vector.tensor_add(a_all, P, xg_t[:, 0:6, :])
            rz = gp.tile([128, 4, BL], BF16, tag=tag + "rz")
            nc.scalar.activation(rz, a_all[:, 0:4, :], AF.Sigmoid)
            rh = gp.tile([128, 2, BL], BF16, tag=tag + "rh")
            nc.vector.tensor_mul(rh, a_all[:, 4:6, :], rz[:, 0:2, :])
            a_n = gp.tile([128, 2, BL], BF16, tag=tag + "a_n")
            nc.vector.tensor_add(a_n, rh, xg_t[:, 6:8, :])
            n_sb = gp.tile([128, 2, BL], BF16, tag=tag + "n")
            nc.scalar.activation(n_sb, a_n, AF.Tanh)
            d = gp.tile([128, 2, BL], BF16, tag=tag + "d")
            nc.vector.tensor_sub(d, hprev, n_sb)
            zd = gp.tile([128, 2, BL], BF16, tag=tag + "zd")
            nc.vector.tensor_mul(zd, rz[:, 2:4, :], d)
            nc.vector.tensor_add(hout, zd, n_sb)

        # ---- main pipeline ----
        h1_tail = zeros2[:, :, :]
        h2_prev = zeros2[:, :, :]
        xg1_subs_prev = None
        h1_cur = None
        for w in range(NW + 1):
            if w < NW:
                xin_w = xinp.tile([KIN, T * BL], BF16, tag="xin")
                nc.sync.dma_start(xin_w, xinT_d[:, w * T * BL : (w + 1) * T * BL])
                xg0_subs = emit_xg_gemm0(xin_w)
                h1_cur = h1p.tile([128, NKCH, T, BL], BF16, tag="h1w")
            for t in range(T):
                if w < NW:
                    hprev0 = h1_tail if t == 0 else h1_cur[:, :, t - 1, :]
                    emit_gru_step(
                        "s0", whh0_sb, xg0_subs[t // SPC], t % SPC, hprev0,
                        h1_cur[:, :, t, :],
                    )
                if w > 0:
                    h2_new = h2p.tile([128, NKCH, BL], BF16, tag="h2")
                    emit_gru_step(
                        "s1", whh1_sb, xg1_subs_prev[t // SPC], t % SPC,
                        h2_prev, h2_new,
                    )
                    h2_prev = h2_new
            if w < NW:
                xg1_subs_prev = emit_xg_gemm1(h1_cur)
                h1_tail = h1_cur[:, :, T - 1, :]

        # ---- FC head on the final h2 ----
        Pfc = psg.tile([BL, 1], F32, tag="psg")
        for kc in range(NKCH):
            nc.tensor.matmul(
                Pfc,
                h2_prev[:, kc, :],
                fcW_sb[:, kc : kc + 1],
                start=(kc == 0),
                stop=(kc == NKCH - 1),
            )
        y_sb = gp.tile([BL, 1], F32, tag="y")
        nc.scalar.activation(y_sb, Pfc, AF.Identity, bias=fcb_sb[:, 0:1])
        nc.sync.dma_start(y_d[:, :], y_sb)

    nc.compile()
    return nc


def prep_core_inputs(inputs, core, S=S_FULL):
    """Numpy layout prep for one core's shard (batch rows [32c, 32c+32))."""
    bf = ml_dtypes.bfloat16
    sl = slice(core * BL, (core + 1) * BL)
    x = np.asarray(inputs["x"], np.float32)[sl, :S]  # [BL, S, 4]
    meta = np.asarray(inputs["meta"], np.float32)[sl]  # [BL, 4]
    xin = np.concatenate(
        [x, np.broadcast_to(meta[:, None, :], (BL, S, meta.shape[-1]))], axis=-1
    )  # [BL, S, 8]
    xinT = np.ascontiguousarray(xin.transpose(2, 1, 0)).reshape(KIN, S * BL)

    def whhT(Wname):
        W = np.asarray(inputs[Wname], np.float32)  # [G, H]
        WT = W.T.reshape(NKCH, 128, G).transpose(1, 0, 2)  # [128, NKCH, G]
        return np.ascontiguousarray(WT).astype(bf)

    def bT(b_ih, b_hh):
        # r/z chunks: b_ih + b_hh; n chunks: b_ih only (b_hn goes inside r*(...))
        b = np.asarray(inputs[b_ih], np.float32).copy()
        b[: 2 * H] += np.asarray(inputs[b_hh], np.float32)[: 2 * H]
        return np.ascontiguousarray(b.reshape(NMCH, 128).T).astype(np.float32)

    SPC = 16

    def bfull(b_ih, b_hh):
        b = np.asarray(inputs[b_ih], np.float32).copy()
        b[: 2 * H] += np.asarray(inputs[b_hh], np.float32)[: 2 * H]
        bT = b.reshape(NMCH, 128).T.astype(bf)  # [128, NMCH]
        full = np.broadcast_to(bT[:, :, None, None], (128, NMCH, SPC, BL))
        return np.ascontiguousarray(full).reshape(128, NMCH, SPC * BL)

    def bhn(b_hh):
        b = np.asarray(inputs[b_hh], np.float32)[2 * H :]
        bT = b.reshape(NKCH, 128).T.astype(bf)  # [128, NKCH]
        full = np.broadcast_to(bT[:, None, :, None], (128, SPC, NKCH, BL))
        return np.ascontiguousarray(full).reshape(128, SPC * NKCH * BL)

    wih0T = np.ascontiguousarray(np.asarray(inputs["W_ih0"], np.float32).T).astype(bf)
    fcW = np.asarray(inputs["fc_W"], np.float32).reshape(H)  # [256]
    fcWT = np.ascontiguousarray(fcW.reshape(NKCH, 128).T).astype(bf)
    fcb = np.full((BL, 1), float(np.asarray(inputs["fc_b"]).reshape(-1)[0]), np.float32)

    return {
        "xinT": xinT.astype(bf),
        "wih0T": wih0T,
        "whh0T": whhT("W_hh0"),
        "wih1T": whhT("W_ih1"),
        "whh1T": whhT("W_hh1"),
        "b0T": bT("b_ih0", "b_hh0"),
        "b1T": bT("b_ih1", "b_hh1"),
        "b0hn": bhn("b_hh0"),
        "b1hn": bhn("b_hh1"),
        "b0f": bfull("b_ih0", "b_hh0"),
        "b1f": bfull("b_ih1", "b_hh1"),
        "fcWT": fcWT,
        "fcb": fcb,
    }


_PROGRAM = None


def kernel(**inputs):
    global _PROGRAM
    if _PROGRAM is None:
        _PROGRAM = build_program()
    in_maps = [prep_core_inputs(inputs, c) for c in range(NCORES)]
    res = run_bass_kernel_spmd(_PROGRAM, in_maps, list(range(NCORES))).results
    y = np.concatenate([np.asarray(res[c]["y"], np.float32) for c in range(NCORES)], 0)
    return y.astype(np.float32)



# revision 6
# speedup vs baseline: 49.6499x; 49.6499x over previous
"""Trainium2 Bass kernel for a 2-layer GRU (B=256, S=1024, IN=4+META=4, H=256) + FC head.

Strategy (data-parallel over batch, 8 cores, 32 batch rows each):
  - All tensors on-chip live in a "transposed" layout: partition dim = 128
    hidden/gate units (chunked), free dim = batch (32), so DVE/ACT use all
    128 lanes.
  - Per GRU step: hg^T = W_hh @ h^T via 12 weight-stationary matmuls
    (6 gate-chunks x 2 K-chunks, N=32 moving cols of h^T), accumulating
    into one PSUM tile [128, 6, 32] (fp32).  Weights stream through
    LDWEIGHTS in bf16 (FWL).
  - Input projections xg = W_ih @ x (+ both biases) are computed per
    window of T steps as efficient GEMMs, evacuated PSUM->SBUF (bf16)
    on ScalarE (ACTIVATE Identity) with the bias folded per partition.
  - The two layers' scans run software-pipelined one window apart so their
    serial gate chains interleave on the engines.
  - Everything except PSUM accumulation and the gate adds is bf16.

Host path: the Bass program is compiled once per process into a cached
jax.jit(shard_map(bass_exec)) callable over the 8 cores.  Only the x/meta
shards stream to the device per call (bf16); all weight-derived arrays are
kept device-resident across calls and re-uploaded only when the weight
inputs' hash changes.  Broadcast bias tiles and the meta-over-time rows are
materialized on device instead of being shipped from the host.
"""

import hashlib
import numpy as np
import ml_dtypes
from contextlib import ExitStack

import concourse.bass as bass
import concourse.bacc as bacc
import concourse.tile as tile
import concourse.mybir as mybir

AF = mybir.ActivationFunctionType
BF16 = mybir.dt.bfloat16
F32 = mybir.dt.float32

B = 256
NCORES = 8
BL = B // NCORES  # 32 batch rows per core
S_FULL = 1024
H = 256
G = 3 * H  # 768
KIN = 8  # IN + META
NMCH = G // 128  # 6 gate chunks
NKCH = H // 128  # 2 hidden chunks


def build_program(S=S_FULL, T=64):
    """Build the single-core SPMD Bass program.

    S: sequence length; T: window (steps per xg GEMM); both scans are
    emitted interleaved with scan1 lagging scan0 by one window.
    """
    assert S % T == 0 and (T * BL) % 512 == 0
    NW = S // T
    NCH = (T * BL) // 512  # 512-wide N-chunks per window GEMM
    SPC = 512 // BL  # steps per N-chunk (16)

    nc = bacc.Bacc()

    xT_d = nc.declare_dram_parameter("xT", [4, S * BL], BF16, False)
    metaT_d = nc.declare_dram_parameter("metaT", [4, BL], BF16, False)
    wih0T_d = nc.declare_dram_parameter("wih0T", [KIN, G], BF16, False)
    whh0T_d = nc.declare_dram_parameter("whh0T", [128, NKCH, G], BF16, False)
    wih1T_d = nc.declare_dram_parameter("wih1T", [128, NKCH, G], BF16, False)
    whh1T_d = nc.declare_dram_parameter("whh1T", [128, NKCH, G], BF16, False)
    b0T_d = nc.declare_dram_parameter("b0T", [128, NMCH], F32, False)
    b1T_d = nc.declare_dram_parameter("b1T", [128, NMCH], F32, False)
    b0hnT_d = nc.declare_dram_parameter("b0hnT", [128, NKCH], F32, False)
    b1hnT_d = nc.declare_dram_parameter("b1hnT", [128, NKCH], F32, False)
    fcWT_d = nc.declare_dram_parameter("fcWT", [128, NKCH], BF16, False)
    fcb_d = nc.declare_dram_parameter("fcb", [BL, 1], F32, False)
    y_d = nc.declare_dram_parameter("y", [BL, 1], F32, True)

    evac_ctr = [0]

    with ExitStack() as ctx:
        tc = ctx.enter_context(tile.TileContext(nc))
        consts = ctx.enter_context(tc.tile_pool(name="consts", bufs=1))
        xgp = ctx.enter_context(tc.tile_pool(name="xgp", bufs=2 * NCH))
        h1p = ctx.enter_context(tc.tile_pool(name="h1p", bufs=2))
        gp = ctx.enter_context(tc.tile_pool(name="gp", bufs=4))
        h2p = ctx.enter_context(tc.tile_pool(name="h2p", bufs=4))
        psc = ctx.enter_context(tc.tile_pool(name="psc", bufs=2, space="PSUM"))
        psg = ctx.enter_context(tc.tile_pool(name="psg", bufs=3, space="PSUM"))

        # ---- constants ----
        whh0_sb = consts.tile([128, NKCH, G], BF16)
        nc.sync.dma_start(whh0_sb, whh0T_d[:, :, :])
        whh1_sb = consts.tile([128, NKCH, G], BF16)
        nc.sync.dma_start(whh1_sb, whh1T_d[:, :, :])
        wih1_sb = consts.tile([128, NKCH, G], BF16)
        nc.sync.dma_start(wih1_sb, wih1T_d[:, :, :])
        wih0_sb = consts.tile([KIN, G], BF16)
        nc.sync.dma_start(wih0_sb, wih0T_d[:, :])
        b0_sb = consts.tile([128, NMCH], F32)
        nc.sync.dma_start(b0_sb, b0T_d[:, :])
        b1_sb = consts.tile([128, NMCH], F32)
        nc.sync.dma_start(b1_sb, b1T_d[:, :])
        b0hn_small = consts.tile([128, NKCH], F32)
        nc.sync.dma_start(b0hn_small, b0hnT_d[:, :])
        b1hn_small = consts.tile([128, NKCH], F32)
        nc.sync.dma_start(b1hn_small, b1hnT_d[:, :])
        meta_sb = consts.tile([4, BL], BF16)
        nc.sync.dma_start(meta_sb, metaT_d[:, :])
        fcW_sb = consts.tile([128, NKCH], BF16)
        nc.sync.dma_start(fcW_sb, fcWT_d[:, :])
        fcb_sb = consts.tile([BL, 1], F32)
        nc.sync.dma_start(fcb_sb, fcb_d[:, :])
        zeros2 = consts.tile([128, NKCH, BL], BF16)
        nc.vector.memset(zeros2, 0.0)

        # broadcast-bias tiles, built on device once
        b0hn_sb = consts.tile([128, SPC, NKCH, BL], BF16)
        nc.vector.tensor_copy(
            b0hn_sb,
            b0hn_small[:, :].unsqueeze(1).unsqueeze(3).broadcast_to([128, SPC, NKCH, BL]),
        )
        b1hn_sb = consts.tile([128, SPC, NKCH, BL], BF16)
        nc.vector.tensor_copy(
            b1hn_sb,
            b1hn_small[:, :].unsqueeze(1).unsqueeze(3).broadcast_to([128, SPC, NKCH, BL]),
        )
        b0f_sb = consts.tile([128, NMCH, SPC, BL], BF16)
        nc.vector.tensor_copy(
            b0f_sb,
            b0_sb[:, :].unsqueeze(2).unsqueeze(3).broadcast_to([128, NMCH, SPC, BL]),
        )
        b1f_sb = consts.tile([128, NMCH, SPC, BL], BF16)
        nc.vector.tensor_copy(
            b1f_sb,
            b1_sb[:, :].unsqueeze(2).unsqueeze(3).broadcast_to([128, NMCH, SPC, BL]),
        )

        # xin double buffers: meta in partitions 0:4 prefilled once (compute ops
        # must start on a quarter-partition boundary); x rows DMA'd into 4:8
        # per window.  wih0T rows are ordered [meta, x] to match.
        xin_bufs = []
        for i in range(2):
            xb = consts.tile([KIN, T * BL], BF16, tag=f"xinbuf{i}")
            nc.vector.tensor_copy(
                xb[0:4, :].rearrange("p (t b) -> p t b", b=BL),
                meta_sb[:, :].unsqueeze(1).broadcast_to([4, T, BL]),
            )
            xin_bufs.append(xb)

        def evac(out_ap, psum_ap, bias_ap, bias_bcast_ap):
            """PSUM->SBUF move with bias add, alternating ScalarE/VectorE.
            DVE uses tensor_add with a broadcast-bias constant (TensorScalarPtr
            is rejected by walrus when Tile attaches >1 sync wait)."""
            evac_ctr[0] += 1
            if evac_ctr[0] % 2 == 0:
                nc.scalar.activation(out_ap, psum_ap, AF.Identity, bias=bias_ap)
            else:
                nc.vector.tensor_add(out_ap, psum_ap, bias_bcast_ap)

        def emit_xg_gemm0(xin_w):
            subs = []
            for nch in range(NCH):
                xg_sub = xgp.tile([128, SPC, 8, BL], BF16, tag="xg0")
                nc.vector.tensor_copy(xg_sub[:, :, 4:6, :], b0hn_sb)
                for m in range(NMCH):
                    P = psg.tile([128, SPC, BL], F32, tag="psg")
                    nc.tensor.matmul(
                        P,
                        wih0_sb[:, bass.ts(m, 128)],
                        xin_w[:, bass.ts(nch, 512)],
                        start=True,
                        stop=True,
                    )
                    evac(xg_sub[:, :, m if m < 4 else m + 2, :], P,
                         b0_sb[:, m : m + 1], b0f_sb[:, m, :, :])
                subs.append(xg_sub)
            return subs

        def emit_xg_gemm1(h1win):
            subs = []
            for nch in range(NCH):
                xg_sub = xgp.tile([128, SPC, 8, BL], BF16, tag="xg1")
                nc.vector.tensor_copy(xg_sub[:, :, 4:6, :], b1hn_sb)
                for m in range(NMCH):
                    P = psg.tile([128, SPC, BL], F32, tag="psg")
                    for kc in range(NKCH):
                        nc.tensor.matmul(
                            P,
                            wih1_sb[:, kc, bass.ts(m, 128)],
                            h1win[:, kc, bass.ts(nch, SPC), :],
                            start=(kc == 0),
                            stop=(kc == NKCH - 1),
                        )
                    evac(xg_sub[:, :, m if m < 4 else m + 2, :], P,
                         b1_sb[:, m : m + 1], b1f_sb[:, m, :, :])
                subs.append(xg_sub)
            return subs

        def emit_gru_step(tag, whh_sb, xg_sub, tl, hprev, hout):
            P = psc.tile([128, NMCH, BL], F32, tag="ps" + tag)
            for m in range(NMCH):
                for kc in range(NKCH):
                    nc.tensor.matmul(
                        P[:, m, :],
                        whh_sb[:, kc, bass.ts(m, 128)],
                        hprev[:, kc, :],
                        start=(kc == 0),
                        stop=(kc == NKCH - 1),
                    )
            xg_t = xg_sub[:, tl, :, :]
            # one add covers r/z gate pre-activations AND (hg_n + b_hn)
            a_all = gp.tile([128, 6, BL], BF16, tag=tag + "a_all")
            nc.vector.tensor_add(a_all, P, xg_t[:, 0:6, :])
            rz = gp.tile([128, 4, BL], BF16, tag=tag + "rz")
            nc.scalar.activation(rz, a_all[:, 0:4, :], AF.Sigmoid)
            rh = gp.tile([128, 2, BL], BF16, tag=tag + "rh")
            nc.vector.tensor_mul(rh, a_all[:, 4:6, :], rz[:, 0:2, :])
            a_n = gp.tile([128, 2, BL], BF16, tag=tag + "a_n")
            nc.vector.tensor_add(a_n, rh, xg_t[:, 6:8, :])
            n_sb = gp.tile([128, 2, BL], BF16, tag=tag + "n")
            nc.scalar.activation(n_sb, a_n, AF.Tanh)
            d = gp.tile([128, 2, BL], BF16, tag=tag + "d")
            nc.vector.tensor_sub(d, hprev, n_sb)
            zd = gp.tile([128, 2, BL], BF16, tag=tag + "zd")
            nc.vector.tensor_mul(zd, rz[:, 2:4, :], d)
            nc.vector.tensor_add(hout, zd, n_sb)

        # ---- main pipeline ----
        h1_tail = zeros2[:, :, :]
        h2_prev = zeros2[:, :, :]
        xg1_subs_prev = None
        h1_cur = None
        for w in range(NW + 1):
            if w < NW:
                xin_w = xin_bufs[w % 2]
                nc.sync.dma_start(
                    xin_w[4:8, :], xT_d[:, w * T * BL : (w + 1) * T * BL]
                )
                xg0_subs = emit_xg_gemm0(xin_w)
                h1_cur = h1p.tile([128, NKCH, T, BL], BF16, tag="h1w")
            for t in range(T):
                if w < NW:
                    hprev0 = h1_tail if t == 0 else h1_cur[:, :, t - 1, :]
                    emit_gru_step(
                        "s0", whh0_sb, xg0_subs[t // SPC], t % SPC, hprev0,
                        h1_cur[:, :, t, :],
                    )
                if w > 0:
                    h2_new = h2p.tile([128, NKCH, BL], BF16, tag="h2")
                    emit_gru_step(
                        "s1", whh1_sb, xg1_subs_prev[t // SPC], t % SPC,
                        h2_prev, h2_new,
                    )
                    h2_prev = h2_new
            if w < NW:
                xg1_subs_prev = emit_xg_gemm1(h1_cur)
                h1_tail = h1_cur[:, :, T - 1, :]

        # ---- FC head on the final h2 ----
        Pfc = psg.tile([BL, 1], F32, tag="psg")
        for kc in range(NKCH):
            nc.tensor.matmul(
                Pfc,
                h2_prev[:, kc, :],
                fcW_sb[:, kc : kc + 1],
                start=(kc == 0),
                stop=(kc == NKCH - 1),
            )
        y_sb = gp.tile([BL, 1], F32, tag="y")
        nc.scalar.activation(y_sb, Pfc, AF.Identity, bias=fcb_sb[:, 0:1])
        nc.sync.dma_start(y_d[:, :], y_sb)

    nc.compile()
    return nc


# ---------------------------------------------------------------------------
# Host-side input prep
# ---------------------------------------------------------------------------

_BF = ml_dtypes.bfloat16

WEIGHT_KEYS = [
    "W_ih0", "W_hh0", "b_ih0", "b_hh0",
    "W_ih1", "W_hh1", "b_ih1", "b_hh1",
    "fc_W", "fc_b",
]


def _rep(a):
    """Concatenate NCORES copies of a along axis 0 (replicated weights)."""
    a = np.ascontiguousarray(a)
    return np.ascontiguousarray(
        np.broadcast_to(a[None], (NCORES,) + a.shape)
    ).reshape((NCORES * a.shape[0],) + a.shape[1:])


def prep_weight_arrays(inputs):
    """Weight-side dram parameter arrays, concatenated across the 8 cores."""
    def whhT(Wname):
        W = np.asarray(inputs[Wname], np.float32)  # [G, H]
        WT = W.T.reshape(NKCH, 128, G).transpose(1, 0, 2)  # [128, NKCH, G]
        return np.ascontiguousarray(WT).astype(_BF)

    def bT(b_ih, b_hh):
        # r/z chunks: b_ih + b_hh; n chunks: b_ih only (b_hn goes inside r*(...))
        b = np.asarray(inputs[b_ih], np.float32).copy()
        b[: 2 * H] += np.asarray(inputs[b_hh], np.float32)[: 2 * H]
        return np.ascontiguousarray(b.reshape(NMCH, 128).T).astype(np.float32)

    def bhnT(b_hh):
        b = np.asarray(inputs[b_hh], np.float32)[2 * H :]
        return np.ascontiguousarray(b.reshape(NKCH, 128).T).astype(np.float32)

    # rows reordered [meta, x] to match the on-device xin layout
    w0 = np.asarray(inputs["W_ih0"], np.float32).T  # [IN+META, G], rows 0:4 x, 4:8 meta
    wih0T = np.ascontiguousarray(np.concatenate([w0[4:8], w0[0:4]], axis=0)).astype(_BF)
    fcW = np.asarray(inputs["fc_W"], np.float32).reshape(H)
    fcWT = np.ascontiguousarray(fcW.reshape(NKCH, 128).T).astype(_BF)
    fcb = np.full((BL, 1), float(np.asarray(inputs["fc_b"]).reshape(-1)[0]), np.float32)

    return {
        "wih0T": _rep(wih0T),
        "whh0T": _rep(whhT("W_hh0")),
        "wih1T": _rep(whhT("W_ih1")),
        "whh1T": _rep(whhT("W_hh1")),
        "b0T": _rep(bT("b_ih0", "b_hh0")),
        "b1T": _rep(bT("b_ih1", "b_hh1")),
        "b0hnT": _rep(bhnT("b_hh0")),
        "b1hnT": _rep(bhnT("b_hh1")),
        "fcWT": _rep(fcWT),
        "fcb": _rep(fcb),
    }


def prep_x_arrays(inputs, S=S_FULL):
    """Per-call data arrays (x, meta), concatenated across the 8 cores."""
    x = np.asarray(inputs["x"], np.float32)[:, :S]  # [B, S, 4]
    xT = (
        x.reshape(NCORES, BL, S, 4)
        .transpose(0, 3, 2, 1)
        .reshape(NCORES * 4, S * BL)
        .astype(_BF)
    )
    meta = np.asarray(inputs["meta"], np.float32)
    metaT = (
        meta.reshape(NCORES, BL, 4).transpose(0, 2, 1).reshape(NCORES * 4, BL)
        .astype(_BF)
    )
    return {"xT": xT, "metaT": metaT}


# ---------------------------------------------------------------------------
# Cached jit runner over the 8 cores
# ---------------------------------------------------------------------------

_ST = {}


def _state():
    if _ST:
        return _ST
    import jax
    from jax.experimental.shard_map import shard_map
    from jax.sharding import Mesh, PartitionSpec
    from concourse.bass2jax import (
        _bass_exec_p,
        install_neuronx_cc_hook,
        partition_id_tensor,
    )

    install_neuronx_cc_hook()
    nc = build_program()

    partition_name = nc.partition_id_tensor.name if nc.partition_id_tensor else None
    in_names, out_names, out_avals, zero_shapes = [], [], [], []
    for alloc in nc.m.functions[0].allocations:
        if not isinstance(alloc, mybir.MemoryLocationSet):
            continue
        name = alloc.memorylocations[0].name
        if alloc.kind == "ExternalInput":
            if name != partition_name:
                in_names.append(name)
        elif alloc.kind == "ExternalOutput":
            out_names.append(name)
            shape = tuple(alloc.tensor_shape)
            dtype = mybir.dt.np(alloc.dtype)
            out_avals.append(jax.core.ShapedArray(shape, dtype))
            zero_shapes.append((shape, dtype))
    n_params = len(in_names)
    n_outs = len(out_avals)
    all_in_names = list(in_names) + list(out_names)
    if partition_name is not None:
        all_in_names.append(partition_name)

    def _body(*args):
        operands = list(args)
        if partition_name is not None:
            operands.append(partition_id_tensor())
        outs = _bass_exec_p.bind(
            *operands,
            out_avals=tuple(out_avals),
            in_names=tuple(all_in_names),
            out_names=tuple(out_names),
            lowering_input_output_aliases=(),
            sim_require_finite=True,
            sim_require_nnan=True,
            nc=nc,
        )
        return tuple(outs)

    devices = jax.devices()[:NCORES]
    mesh = Mesh(np.asarray(devices), ("core",))
    in_specs = (PartitionSpec("core"),) * (n_params + n_outs)
    out_specs = (PartitionSpec("core"),) * n_outs
    sharded = jax.jit(
        shard_map(
            _body, mesh=mesh, in_specs=in_specs, out_specs=out_specs, check_rep=False
        ),
        donate_argnums=tuple(range(n_params, n_params + n_outs)),
        keep_unused=True,
    )
    _ST.update(
        nc=nc, sharded=sharded, mesh=mesh, in_names=in_names,
        out_names=out_names, zero_shapes=zero_shapes, jax=jax,
    )
    return _ST


def _weights_hash(inputs):
    h = hashlib.blake2b(digest_size=16)
    for k in WEIGHT_KEYS:
        h.update(np.ascontiguousarray(np.asarray(inputs[k], np.float32)).tobytes())
    return h.hexdigest()


def kernel(**inputs):
    st = _state()
    jax = st["jax"]
    from jax.sharding import NamedSharding, PartitionSpec

    hsh = _weights_hash(inputs)
    if st.get("whash") != hsh:
        wd = prep_weight_arrays(inputs)
        shard = NamedSharding(st["mesh"], PartitionSpec("core"))
        st["wdev"] = {k: jax.device_put(v, shard) for k, v in wd.items()}
        jax.block_until_ready(list(st["wdev"].values()))
        st["whash"] = hsh

    xa = prep_x_arrays(inputs)
    args = [
        xa[nm] if nm in xa else st["wdev"][nm] for nm in st["in_names"]
    ]
    zeros = [
        np.zeros((NCORES * s[0],) + tuple(s[1:]), d) for s, d in st["zero_shapes"]
    ]
    outs = st["sharded"](*args, *zeros)
    y = np.asarray(outs[st["out_names"].index("y")], np.float32)
    return y.reshape(B, 1)
